# revision 9
# baseline (speedup 1.0000x reference)
"""Coordinate-descent (alternating Gauss-Seidel) kernel for Trainium2, v6.

B=4 factorizations x ~ u @ v^T, M=N=4096, R=32, row-sharded over 8 cores.

v6 changes vs v5:
 - All-f32r datapath: f32r is a bit-reinterpret of f32, and the PE streams
   it at the same 1 col/cycle as bf16, so the whole f32->bf16 cast pass
   (DVE) and the bf16 staging copies are deleted.  x is DMA'd straight
   into its SBUF stream tile.
 - The 5-step nilpotent solve chain is collapsed into ONE matmul: the
   32x32 matrix G^ = (M^{-1} D'^{-1})^T is precomputed per Gram from the
   same W-power tiles, so each half-solve is one DVE epilogue + one
   512-col MM instead of 5 dependent MMs + 5 DVE ops.
 - ReduceScatter payloads in bf16 (half the collective time).
 - DMA issue is split across the two HWDGE queues (x stream on sync,
   everything else on scalar) to fix the serial-issue startup stall.
 - v-solve of batch b runs during batch b+1 (was b+3); the last batch's
   RS is split in two column halves so its v-solve overlaps the second
   half's collective.
 - a1 consumption MMs are deferred two 512-col groups behind the
   transposes so the PE queue never stalls on the v loads.
"""

import os
from contextlib import ExitStack

import numpy as np

import concourse.bass as bass
import concourse.tile as tile
from concourse import bacc, mybir
from concourse.bass import ds
from concourse.bass_utils import run_bass_kernel_spmd
from concourse.masks import make_identity, make_lower_triangular

B, M, N, R = 4, 4096, 4096, 32
NCORES = 8
MS = M // NCORES          # 512 rows per core per batch
MC = MS // 128            # 4 m-chunks of 128
NG = N // 512             # 8 n-groups of 512
NCH = N // 128            # 32 n-chunks of 128
EPS = 1e-8
F32 = mybir.dt.float32
F32R = mybir.dt.float32r
BF16 = mybir.dt.bfloat16
ALU = mybir.AluOpType
AX = mybir.AxisListType

_CACHE = {}
LAST_RESULT = None


def _gram_prep(nc, smp, pwp, punp, consts, b_sb, tg):
    """From Gram b (SBUF f32r) precompute nbsl and ghat = (M^{-1}D'^{-1})^T.

    M^T = D' + bsl (b symmetric), W = D'^{-1} bsl strictly lower,
    (I+W)^{-1} = (I-W)(I+W^2)(I+W^4)(I+W^8)(I+W^16) exactly (W^32 = 0).
    ghat = D'^{-1} G with G = [(I+W)^{-1}]^T built by the reversed
    transposed chain.  The solve is then u_new^T = ghat.T @ (rhs + eps).
    """
    ident32_r, masksl_r, eye_r = consts

    bd = smp.tile([R, R], F32R, tag=f"bd{tg}", name="bd", bufs=1)
    nc.vector.tensor_tensor(out=bd[:], in0=b_sb[:], in1=eye_r, op=ALU.mult)
    d_p = smp.tile([R, 1], F32, tag=f"dp{tg}", name="d_p", bufs=1)
    inv_p = smp.tile([R, 1], F32, tag=f"ip{tg}", name="inv_p", bufs=1)
    nc.vector.tensor_reduce(d_p[:], bd[:], axis=AX.X, op=ALU.add)
    nc.vector.tensor_scalar_add(inv_p[:], d_p[:], EPS)
    nc.vector.reciprocal(inv_p[:], inv_p[:])
    invb = bass.AP(inv_p[:].tensor, inv_p[:].offset, [inv_p[:].ap[0], [0, R]])

    bsl = smp.tile([R, R], F32R, tag=f"bsl{tg}", name="bsl", bufs=1)
    nc.vector.tensor_tensor(out=bsl[:], in0=b_sb[:], in1=masksl_r,
                            op=ALU.mult)
    nbsl = smp.tile([R, R], F32R, tag=f"nbsl{tg}", name="nbsl", bufs=1)
    nc.vector.tensor_scalar_mul(nbsl[:], bsl[:], -1.0)
    vw = smp.tile([R, R], F32R, tag=f"vw{tg}", name="vw", bufs=1)  # W
    nc.vector.tensor_tensor(out=vw[:], in0=bsl[:], in1=invb, op=ALU.mult)

    # transpose W -> W^T (f32r out must live in the f32r pun slot)
    pwt = punp.tile([128, MC, R], F32R, tag="pun", name="pwt")
    nc.tensor.transpose(pwt[:R, 0, :], vw[:], ident32_r)
    w1 = smp.tile([R, R], F32R, tag=f"w1{tg}", name="w1", bufs=1)
    nc.scalar.copy(w1[:], pwt[:R, 0, :])

    def _mm_small(lhsT, rhs, tagn):
        p = pwp.tile([R, R], F32, tag="pw", name="pmm")
        nc.tensor.matmul(p[:], lhsT=lhsT[:], rhs=rhs[:], start=True,
                         stop=True)
        s = smp.tile([R, R], F32R, tag=f"{tagn}{tg}", name=tagn, bufs=1)
        nc.scalar.copy(s[:], p[:])
        return s

    # powers of W: plain tiles are (W^k)^T, "t" tiles are W^k
    w2 = _mm_small(vw, w1, "w2")      # vw.T @ w1  = (W^2)^T
    w2t = _mm_small(w1, vw, "w2t")    # w1.T @ vw  = W^2
    w4 = _mm_small(w2t, w2, "w4")     # (W^4)^T
    w4t = _mm_small(w2, w2t, "w4t")   # W^4
    w8 = _mm_small(w4t, w4, "w8")     # (W^8)^T
    w8t = _mm_small(w4, w4t, "w8t")   # W^8
    w16t = _mm_small(w8, w8t, "w16t")  # W^16

    # G chain: G0 = I - W^T; G <- G + (W^{2^k})^T G  (lhsT = W^{2^k})
    g = smp.tile([R, R], F32R, tag=f"g0{tg}", name="g0", bufs=1)
    nc.vector.tensor_tensor(out=g[:], in0=ident32_r, in1=w1[:],
                            op=ALU.subtract)
    for k, wkt in enumerate((w2t, w4t, w8t, w16t)):
        pg = pwp.tile([R, R], F32, tag="pw", name="pg")
        nc.tensor.matmul(pg[:], lhsT=wkt[:], rhs=g[:], start=True, stop=True)
        gn = smp.tile([R, R], F32R, tag=f"g{k + 1}{tg}", name="gn", bufs=1)
        nc.vector.tensor_tensor(out=gn[:], in0=g[:], in1=pg[:], op=ALU.add)
        g = gn
    ghat = smp.tile([R, R], F32R, tag=f"gh{tg}", name="ghat", bufs=1)
    nc.vector.tensor_tensor(out=ghat[:], in0=g[:], in1=invb, op=ALU.mult)

    return {"nbsl": nbsl, "ghat": ghat}


def _build():
    nc = bacc.Bacc("TRN2", target_bir_lowering=False, debug=False,
                   num_devices=NCORES)

    x_my = nc.dram_tensor("x_my", [B, MS, N], F32, kind="ExternalInput").ap()
    u_my = nc.dram_tensor("u_my", [B, MS, R], F32, kind="ExternalInput").ap()
    v_full = nc.dram_tensor("v_full", [B, N, R], F32,
                            kind="ExternalInput").ap()
    v_my = nc.dram_tensor("v_my", [B, MS, R], F32, kind="ExternalInput").ap()
    u_out = nc.dram_tensor("u_out", [B, MS, R], F32,
                           kind="ExternalOutput").ap()
    v_out = nc.dram_tensor("v_out", [B, MS, R], F32,
                           kind="ExternalOutput").ap()

    # bf16 collective payloads; batch 3's RS is split into two halves:
    # half a = a2T cols 0:256 + b2, half b = a2T cols 256:512.
    rs_ins = [nc.dram_tensor(f"rs_in_{b}", [NCORES * R, 512 + R], BF16)
              for b in range(B - 1)]
    rs_outs = [nc.dram_tensor(f"rs_out_{b}", [R, 512 + R], BF16)
               for b in range(B - 1)]
    rs_in_3a = nc.dram_tensor("rs_in_3a", [NCORES * R, 256 + R], BF16)
    rs_out_3a = nc.dram_tensor("rs_out_3a", [R, 256 + R], BF16)
    rs_in_3b = nc.dram_tensor("rs_in_3b", [NCORES * R, 256], BF16)
    rs_out_3b = nc.dram_tensor("rs_out_3b", [R, 256], BF16)

    with tile.TileContext(nc) as tc, ExitStack() as ctx:
        const = ctx.enter_context(tc.tile_pool(name="const", bufs=1))
        xbp = ctx.enter_context(tc.tile_pool(name="xbp", bufs=1))
        xtp = ctx.enter_context(tc.tile_pool(name="xtp", bufs=8))
        vp = ctx.enter_context(tc.tile_pool(name="vp", bufs=2))
        smp = ctx.enter_context(tc.tile_pool(name="smp", bufs=2))
        a2sp = ctx.enter_context(tc.tile_pool(name="a2sp", bufs=2))
        # PSUM banks: ppt 2 + pa1 1 + pa2 2 + pw 1 + pun 1 + psol 1 = 8
        ppt = ctx.enter_context(tc.tile_pool(name="ppt", bufs=2,
                                             space="PSUM"))
        pa1p = ctx.enter_context(tc.tile_pool(name="pa1", bufs=1,
                                              space="PSUM"))
        pa2p = ctx.enter_context(tc.tile_pool(name="pa2", bufs=1,
                                              space="PSUM"))
        pwp = ctx.enter_context(tc.tile_pool(name="pw", bufs=1,
                                             space="PSUM"))
        punp = ctx.enter_context(tc.tile_pool(name="pun", bufs=1,
                                              space="PSUM"))
        psolp = ctx.enter_context(tc.tile_pool(name="psol", bufs=2,
                                               space="PSUM"))

        ident128_f = const.tile([128, 128], F32)
        make_identity(nc, ident128_f)
        ident128_r = const.tile([128, 128], F32R)
        nc.vector.tensor_copy(ident128_r[:], ident128_f[:])
        ident32_f = const.tile([R, R], F32)
        make_identity(nc, ident32_f)
        masksl_f = const.tile([R, R], F32)
        make_lower_triangular(nc, masksl_f, val=1.0, diag=False)
        ident32_r = const.tile([R, R], F32R)
        nc.vector.tensor_copy(ident32_r[:], ident32_f[:])
        masksl_r = const.tile([R, R], F32R)
        nc.vector.tensor_copy(masksl_r[:], masksl_f[:])
        consts = (ident32_r[:], masksl_r[:], ident32_r[:])

        # ---------- x(b0) first groups on sync, then v loads on scalar ----
        xbs = {}

        def xb_tile(b):
            t = xbp.tile([128, MC, N], F32R, tag=f"xb{b % 2}", name="xb")
            xbs[b] = t
            return t

        xb0 = xb_tile(0)
        x_re0 = x_my[0].rearrange("(i p) n -> p i n", p=128).bitcast(F32R)
        NPREF = 2
        for g in range(NPREF):
            nc.sync.dma_start(xb0[:, :, g * 512:(g + 1) * 512],
                              x_re0[:, :, g * 512:(g + 1) * 512])

        vts = []
        for b in range(B):
            v32 = vp.tile([128, NCH, R], F32R, tag=f"v32{b}", name="v32",
                          bufs=1)
            nc.scalar.dma_start(
                v32[:],
                v_full[b].rearrange("(c p) r -> p c r", p=128).bitcast(F32R))
            vts.append(v32)

        u_preps = {}

        def emit_prep(b):
            pb1 = pwp.tile([R, R], F32, tag="pw", name="pb1")
            for j in range(NCH):
                nc.tensor.matmul(pb1[:], lhsT=vts[b][:, j, :],
                                 rhs=vts[b][:, j, :], start=(j == 0),
                                 stop=(j == NCH - 1), skip_group_check=True)
            b1_sb = smp.tile([R, R], F32R, tag=f"b1s{b}", name="b1_sb",
                             bufs=1)
            nc.scalar.copy(b1_sb[:], pb1[:])
            u_preps[b] = _gram_prep(nc, smp, pwp, punp, consts, b1_sb,
                                    f"u{b}")

        state = {}

        def back_transpose(zout, cols, out32, tg):
            """zout [R, len(cols)*128] (SBUF) -> out32 chunks via PE."""
            pun = punp.tile([128, MC, R], F32R, tag="pun", name=f"pun{tg}")
            for k, i in enumerate(cols):
                nc.tensor.transpose(pun[:, i, :],
                                    zout[:, k * 128:(k + 1) * 128],
                                    ident32_r[:])
            for k, i in enumerate(cols):
                nc.scalar.copy(out32[:, i, :], pun[:, i, :])

        def emit_solve_v(b):
            st = state[b]
            prep = st["vprep"]
            vn32 = smp.tile([128, MC, R], F32R, tag="vn32", name="vn32", bufs=1)
            # ps = -bsl^T vT  (shared by both halves)
            ps = psolp.tile([R, MS], F32, tag="psol", name="ps")
            nc.tensor.matmul(ps[:], lhsT=prep["nbsl"][:], rhs=st["vT"][:],
                             start=True, stop=True)
            for h, (a2t, c0, c1) in enumerate(st["a2t_halves"]):
                a2t32 = smp.tile([R, 256], F32R, tag=f"a2t32{h}",
                                 name="a2t32", bufs=1)
                nc.scalar.copy(a2t32[:], a2t)
                zin = smp.tile([R, 256], F32R, tag=f"zinv{h}", name="zin", bufs=1)
                nc.vector.scalar_tensor_tensor(
                    out=zin[:], in0=ps[:, c0:c1], scalar=EPS,
                    in1=a2t32[:], op0=ALU.add, op1=ALU.add)
                pz = psolp.tile([R, 256], F32, tag="psol", name=f"pz{h}")
                nc.tensor.matmul(pz[:], lhsT=prep["ghat"][:], rhs=zin[:],
                                 start=True, stop=True,
                                 skip_group_check=True)
                zout = smp.tile([R, 256], F32R, tag=f"zov{h}", name="zout", bufs=1)
                nc.scalar.copy(zout[:], pz[:])
                back_transpose(zout, (2 * h, 2 * h + 1), vn32, f"v{h}")
            nc.scalar.dma_start(
                v_out[b].rearrange("(i p) r -> p i r", p=128).bitcast(F32R),
                vn32[:])

        def emit_prep_v(b):
            st = state[b]
            b2bf, = st["b2_tiles"]
            b2_sb = smp.tile([R, R], F32R, tag="b2s", name="b2_sb", bufs=1)
            nc.scalar.copy(b2_sb[:], b2bf)
            st["vprep"] = _gram_prep(nc, smp, pwp, punp, consts, b2_sb,
                                     f"v{b}")

        p2state = {}

        def emit_phase2_part(bp, gp):
            # one 512-col group of the a2T accumulation for batch bp
            # (no tile_position packing: dst partition offsets != 0 are
            # ISA-illegal for 4-byte matmuls)
            un_p, xb_p = p2state[bp]
            pa2 = pa2p.tile([R, MS], F32, tag="pa2", name="pa2")
            for i in range(MC):
                nc.tensor.matmul(
                    pa2[:], lhsT=un_p[:, i, :],
                    rhs=xb_p[:, i, gp * 512:(gp + 1) * 512],
                    start=(i == 0), stop=(i == MC - 1),
                    skip_group_check=True)
            a2st = a2sp.tile([R, MS], BF16, tag="a2st", name="a2st")
            nc.vector.tensor_copy(a2st[:], pa2[:])
            if bp < B - 1:
                nc.scalar.dma_start(
                    rs_ins[bp].ap()[ds(gp * R, R), 0:512], a2st[:])
            else:
                nc.scalar.dma_start(
                    rs_in_3a.ap()[ds(gp * R, R), 0:256], a2st[:, 0:256])
                nc.scalar.dma_start(
                    rs_in_3b.ap()[ds(gp * R, R), 0:256], a2st[:, 256:512])

        def bcast_b2(b2st, rs_dram, col0):
            # one DMA writing all 8 replicas of b2: out [r, c, k]
            out_ap = rs_dram.ap()[:, col0:col0 + R].rearrange(
                "(c r) k -> r c k", c=NCORES)
            src = b2st[:]
            in_ap = bass.AP(src.tensor, src.offset,
                            [src.ap[0], [0, NCORES], src.ap[1]])
            nc.scalar.dma_start(out_ap, in_ap)

        def emit_b2(bp):
            un_p, _ = p2state[bp]
            pb2 = pwp.tile([R, R], F32, tag="pw", name="pb2")
            for i in range(MC):
                nc.tensor.matmul(pb2[:], lhsT=un_p[:, i, :],
                                 rhs=un_p[:, i, :], start=(i == 0),
                                 stop=(i == MC - 1), skip_group_check=True)
            b2st = a2sp.tile([R, R], BF16, tag="b2st", name="b2st")
            nc.scalar.copy(b2st[:], pb2[:])
            if bp < B - 1:
                bcast_b2(b2st, rs_ins[bp], 512)
            else:
                bcast_b2(b2st, rs_in_3a, 256)

        def emit_rs(bp):
            nc.gpsimd.collective_compute(
                "ReduceScatter", ALU.add,
                replica_groups=[list(range(NCORES))],
                ins=[rs_ins[bp].ap()], outs=[rs_outs[bp].ap()])

        def load_rs_result(bp):
            st = state[bp]
            if bp < B - 1:
                a2t = smp.tile([R, MS], BF16, tag="a2t", name="a2t", bufs=1)
                nc.scalar.dma_start(a2t[:], rs_outs[bp].ap()[:, 0:512])
                b2bf = smp.tile([R, R], BF16, tag="b2bf", name="b2bf", bufs=1)
                nc.scalar.dma_start(b2bf[:],
                                    rs_outs[bp].ap()[:, 512:512 + R])
                st["a2t_halves"] = [(a2t[:, 0:256], 0, 256),
                                    (a2t[:, 256:512], 256, 512)]
            else:
                a2ta = smp.tile([R, 256], BF16, tag="a2t", name="a2ta", bufs=1)
                nc.scalar.dma_start(a2ta[:], rs_out_3a.ap()[:, 0:256])
                b2bf = smp.tile([R, R], BF16, tag="b2bf", name="b2bf", bufs=1)
                nc.scalar.dma_start(b2bf[:],
                                    rs_out_3a.ap()[:, 256:256 + R])
                a2tb = smp.tile([R, 256], BF16, tag="a2tb", name="a2tb", bufs=1)
                nc.scalar.dma_start(a2tb[:], rs_out_3b.ap()[:])
                st["a2t_halves"] = [(a2ta[:], 0, 256), (a2tb[:], 256, 512)]
            st["b2_tiles"] = (b2bf[:],)

        # ================= main batch loop =================
        for b in range(B):
            xb = xbs[b]
            x_re = x_my[b].rearrange("(i p) n -> p i n", p=128).bitcast(F32R)

            # per-batch u/v row-shard loads (uT/vT transposes emitted at g2)
            u32 = vp.tile([128, MC, R], F32R, tag="u32", name="u32")
            nc.scalar.dma_start(
                u32[:],
                u_my[b].rearrange("(i p) r -> p i r", p=128).bitcast(F32R))
            vm32 = vp.tile([128, MC, R], F32R, tag="vm32", name="vm32")
            nc.scalar.dma_start(
                vm32[:],
                v_my[b].rearrange("(i p) r -> p i r", p=128).bitcast(F32R))
            uvT = {}

            def emit_uvT():
                put = psolp.tile([R, MS], F32R, tag="psol", name="put")
                for i in range(MC):
                    nc.tensor.transpose(put[:, i * 128:(i + 1) * 128],
                                        u32[:, i, :], ident128_r[:])
                uT = smp.tile([R, MS], F32R, tag="uT", name="uT", bufs=1)
                nc.scalar.copy(uT[:], put[:])
                pvt = psolp.tile([R, MS], F32R, tag="psol", name="pvt")
                for i in range(MC):
                    nc.tensor.transpose(pvt[:, i * 128:(i + 1) * 128],
                                        vm32[:, i, :], ident128_r[:])
                vT = smp.tile([R, MS], F32R, tag="vT", name="vT", bufs=2)
                nc.scalar.copy(vT[:], pvt[:])
                uvT["uT"], uvT["vT"] = uT, vT

            # ---------------- phase 1: stream x ----------------
            pa1 = pa1p.tile([R, MS], F32, tag="pa1", name="pa1")
            DEFER = 2

            def emit_a1(g):
                # a1^T accumulation for group g's four n-chunks
                for j2 in range(4):
                    j = 4 * g + j2
                    xt = p1xt[j]
                    nc.tensor.matmul(pa1[:], lhsT=vts[b][:, j, :],
                                     rhs=xt.rearrange("p a b -> p (a b)"),
                                     start=(j == 0), stop=False,
                                     skip_group_check=True)

            p1xt = {}
            for g in range(NG):
                if not (b > 0 and g < NPREF):
                    nc.sync.dma_start(xb[:, :, g * 512:(g + 1) * 512],
                                      x_re[:, :, g * 512:(g + 1) * 512])
                for j2 in range(4):
                    j = 4 * g + j2
                    pt = ppt.tile([128, MC, 128], F32R, tag="pt", name="pt")
                    for i in range(MC):
                        nc.tensor.transpose(
                            pt[:, i], xb[:, i, j * 128:(j + 1) * 128],
                            ident128_r[:])
                    xt = xtp.tile([128, MC, 128], F32R, tag="xt", name="xt")
                    nc.vector.tensor_copy(xt[:], pt[:])
                    p1xt[j] = xt
                if g >= DEFER:
                    emit_a1(g - DEFER)
                # slot work overlapped with the stream
                if g == 2:
                    emit_uvT()
                if b == 0 and 1 <= g <= B:
                    emit_prep(g - 1)
                if b >= 1:
                    if g == 0:
                        for gp in range(4):
                            emit_phase2_part(b - 1, gp)
                    elif g == 1:
                        for gp in range(4, 8):
                            emit_phase2_part(b - 1, gp)
                        emit_b2(b - 1)
                        emit_rs(b - 1)
                    elif g == 5:
                        load_rs_result(b - 1)
                    elif g == 7:
                        emit_prep_v(b - 1)
            for g in range(NG - DEFER, NG):
                emit_a1(g)

            # prefetch next batch's first groups
            if b + 1 < B:
                xb_n = xb_tile(b + 1)
                x_re_n = (x_my[b + 1]
                          .rearrange("(i p) n -> p i n", p=128)
                          .bitcast(F32R))
                for g in range(NPREF):
                    nc.sync.dma_start(xb_n[:, :, g * 512:(g + 1) * 512],
                                      x_re_n[:, :, g * 512:(g + 1) * 512])

            # ---------------- u solve (s-MM fused into pa1 group) --------
            uT, vT = uvT["uT"], uvT["vT"]
            nc.tensor.matmul(pa1[:], lhsT=u_preps[b]["nbsl"][:], rhs=uT[:],
                             start=False, stop=True, skip_group_check=True)
            zin = smp.tile([R, MS], F32R, tag="zinu", name="zinu", bufs=1)
            nc.vector.tensor_scalar_add(zin[:], pa1[:], EPS)
            pzu = psolp.tile([R, MS], F32, tag="psol", name="pzu")
            nc.tensor.matmul(pzu[:], lhsT=u_preps[b]["ghat"][:], rhs=zin[:],
                             start=True, stop=True, skip_group_check=True)
            zout = smp.tile([R, MS], F32R, tag="zou", name="zou", bufs=1)
            nc.scalar.copy(zout[:], pzu[:])
            un32 = smp.tile([128, MC, R], F32R, tag="un32", name="un32", bufs=1)
            back_transpose(zout, range(MC), un32, "u")
            nc.scalar.dma_start(
                u_out[b].rearrange("(i p) r -> p i r", p=128).bitcast(F32R),
                un32[:])

            p2state[b] = (un32, xb)
            state[b] = {"vT": vT}

            # v-solve of the previous batch (RS had ~a full stream to run)
            if b >= 1:
                emit_solve_v(b - 1)

        # ================= tail: last batch's phase 2 =================
        bl = B - 1
        emit_b2(bl)
        for gp in range(8):
            emit_phase2_part(bl, gp)
        nc.gpsimd.collective_compute(
            "ReduceScatter", ALU.add,
            replica_groups=[list(range(NCORES))],
            ins=[rs_in_3a.ap()], outs=[rs_out_3a.ap()])
        nc.gpsimd.collective_compute(
            "ReduceScatter", ALU.add,
            replica_groups=[list(range(NCORES))],
            ins=[rs_in_3b.ap()], outs=[rs_out_3b.ap()])
        load_rs_result(bl)
        emit_prep_v(bl)
        emit_solve_v(bl)

    nc.compile()
    return nc


def kernel(x, u, v):
    global LAST_RESULT
    if "nc" not in _CACHE:
        _CACHE["nc"] = _build()
    nc = _CACHE["nc"]

    x = np.ascontiguousarray(x, dtype=np.float32)
    u = np.ascontiguousarray(u, dtype=np.float32)
    v = np.ascontiguousarray(v, dtype=np.float32)

    in_maps = []
    for c in range(NCORES):
        sl = slice(c * MS, (c + 1) * MS)
        in_maps.append({
            "x_my": np.ascontiguousarray(x[:, sl, :]),
            "u_my": np.ascontiguousarray(u[:, sl, :]),
            "v_full": v,
            "v_my": np.ascontiguousarray(v[:, sl, :]),
        })

    res = run_bass_kernel_spmd(nc, in_maps, list(range(NCORES)),
                               trace=os.environ.get("KBENCH_TRACE") == "1")
    LAST_RESULT = res
    u_new = np.concatenate([res.results[c]["u_out"] for c in range(NCORES)],
                           axis=1)
    v_new = np.concatenate([res.results[c]["v_out"] for c in range(NCORES)],
                           axis=1)
    return (u_new, v_new)
